# revision 14
# baseline (speedup 1.0000x reference)
"""GroupedQueryAttention forward on 8 Trainium2 NeuronCores (Bass/Tile).

Sharding (per spec hint): data-parallel over batch (B=2) x tensor-parallel
over KV-head groups (4 groups of 2 KV heads + their 8 query heads each).
Core c -> (batch b = c // 4, group g = c % 4).

Each core computes, for its batch element and its 8 query heads:
  qT/kT projections in transposed layout (lhsT = W, rhs = xT), V natural via
  on-chip PE transpose of vT; causal softmax without max-subtraction (scores
  are ~N(0,1) after the 1/sqrt(hd) scale, exp cannot overflow).  The V tiles
  carry a 64-wide ones block so the same attn@V matmul emits the softmax
  denominator replicated across 64 partitions; the normalize is then an
  approx reciprocal (single DVE op) plus an elementwise multiply -- no PE
  broadcast.  The causal diagonal runs at 128-query granularity.  All inputs
  arrive pre-tiled from the host so every DMA is contiguous per partition,
  and the schedule interleaves per-chunk K/V/Q projections, attention, and
  o_proj so every engine starts early and the PE never drains.  o_proj is
  row-parallel: each core emits a full [N, D] fp32 partial and the host sums
  the 4 partials per batch element (the o_proj "all-reduce").

All device compute is bf16 with fp32 PSUM accumulation.
"""

import numpy as np

import concourse.bass as bass  # noqa: F401  (import keeps engine registry warm)
import concourse.mybir as mybir
import concourse.tile as tile
from concourse import bacc, bass_utils

# Problem shape (hardcoded per contract).
B, N, D = 2, 2048, 2048
NUM_HEADS = 32
NUM_KV_HEADS = 8
HD = 64                      # head dim
G = NUM_HEADS // NUM_KV_HEADS  # 4 query heads per kv head
N_CORES = 8
NT = D // 128                # 16 contraction tiles
NCHUNK = 4                   # token chunks of 512
CH = 512

_CACHE = {}


def _build():
    nc = bacc.Bacc("TRN2", target_bir_lowering=False, debug=False,
                   num_devices=N_CORES)
    f32, bf16 = mybir.dt.float32, mybir.dt.bfloat16

    # All inputs pre-tiled host-side to [128, ...] SBUF layout (contiguous
    # per-partition DMA).  x is chunk-major: [128, (chunk, ktile, 512)].
    xT = nc.dram_tensor("xT", [128, NCHUNK * NT * CH], bf16,
                        kind="ExternalInput")
    wq = nc.dram_tensor("wq", [128, NT * 512], bf16, kind="ExternalInput")
    wk = nc.dram_tensor("wk", [128, NT * 128], bf16, kind="ExternalInput")
    wv = nc.dram_tensor("wv", [128, NT * 128], bf16, kind="ExternalInput")
    wo = nc.dram_tensor("wo", [128, 4 * D], bf16, kind="ExternalInput")
    msk = nc.dram_tensor("msk", [128, 128], bf16, kind="ExternalInput")
    iden = nc.dram_tensor("iden", [128, 128], bf16, kind="ExternalInput")
    part = nc.dram_tensor("part", [N, D], f32, kind="ExternalOutput")

    def xcol(j, t):
        return (j * NT + t) * CH

    with tile.TileContext(nc) as tc:
        with (
            tc.tile_pool(name="const", bufs=1) as cpool,
            tc.tile_pool(name="proj", bufs=1) as ppool,
            tc.tile_pool(name="work", bufs=4) as wpool,
            tc.tile_pool(name="att", bufs=1) as apool,
            tc.tile_pool(name="stage", bufs=3) as spool,
            tc.tile_pool(name="ps", bufs=1, space="PSUM") as ps,
        ):
            # ---- DMA loads, ordered so compute can chase arrival ---------
            msk_t = cpool.tile([128, 128], bf16, tag="msk")
            nc.sync.dma_start(msk_t[:], msk.ap()[:])
            id_t = cpool.tile([128, 128], bf16, tag="iden")
            nc.sync.dma_start(id_t[:], iden.ap()[:])
            wk_t = cpool.tile([128, NT * 128], bf16, tag="wk")
            nc.sync.dma_start(wk_t[:], wk.ap()[:])
            wv_t = cpool.tile([128, NT * 128], bf16, tag="wv")
            nc.sync.dma_start(wv_t[:], wv.ap()[:])
            xt = cpool.tile([128, NCHUNK * NT * CH], bf16, tag="xt")
            for q4 in range(4):
                nc.sync.dma_start(
                    xt[:, q4 * 4 * CH:(q4 + 1) * 4 * CH],
                    xT.ap()[:, q4 * 4 * CH:(q4 + 1) * 4 * CH])
            wq_t = cpool.tile([128, NT * 512], bf16, tag="wq")
            nc.sync.dma_start(wq_t[:], wq.ap()[:])
            for j in range(1, NCHUNK):
                nc.sync.dma_start(xt[:, j * NT * CH:(j + 1) * NT * CH],
                                  xT.ap()[:, j * NT * CH:(j + 1) * NT * CH])
            wo_t = cpool.tile([128, 4 * D], bf16, tag="wo")
            nc.sync.dma_start(wo_t[:], wo.ap()[:])

            # ---- persistent activation tiles ------------------------------
            # kt2 [128 (2 kv heads x 64), N]; v3 m-tile layout per 256 cols:
            # [V_kv0(64) | ones(64) | V_kv1(64) | ones(64)]
            kt2 = ppool.tile([128, N], bf16, tag="kt2")
            v3 = apool.tile([128, 16 * 256], bf16, tag="v3")
            nc.vector.memset(v3[:], 1.0)
            qt2 = [ppool.tile([128, N], bf16, tag=f"qt2_{a}", name=f"qt2_{a}")
                   for a in range(4)]

            def kv_chunk(j):
                psk = ps.tile([128, CH], f32, tag="mm", bufs=2, name="psk")
                for t in range(NT):
                    nc.tensor.matmul(
                        psk[:], wk_t[:, t * 128:(t + 1) * 128],
                        xt[:, xcol(j, t):xcol(j, t) + CH],
                        start=(t == 0), stop=(t == NT - 1))
                nc.scalar.activation(kt2[:, j * CH:(j + 1) * CH], psk[:],
                                     mybir.ActivationFunctionType.Copy)
                psv = ps.tile([128, CH], f32, tag="mm", bufs=2, name="psv")
                for t in range(NT):
                    nc.tensor.matmul(
                        psv[:], wv_t[:, t * 128:(t + 1) * 128],
                        xt[:, xcol(j, t):xcol(j, t) + CH],
                        start=(t == 0), stop=(t == NT - 1))
                vt_s = spool.tile([128, CH], bf16, tag="vt")
                nc.scalar.activation(vt_s[:], psv[:],
                                     mybir.ActivationFunctionType.Copy)
                for s in range(4):       # 4 m-tiles of 128 in this chunk
                    mt = 4 * j + s
                    pst = ps.tile([128, 128], bf16, tag="s", bufs=4,
                                  name="pst")
                    nc.tensor.transpose(pst[:], vt_s[:, s * 128:(s + 1) * 128],
                                        id_t[:])
                    nc.vector.tensor_copy(v3[:, mt * 256: mt * 256 + 64],
                                          pst[:, 0:64])
                    nc.vector.tensor_copy(v3[:, mt * 256 + 128: mt * 256 + 192],
                                          pst[:, 64:128])

            def q_chunk(j):
                for a in range(4):
                    psq = ps.tile([128, CH], f32, tag="mm", bufs=2,
                                  name="psq")
                    for t in range(NT):
                        nc.tensor.matmul(
                            psq[:],
                            wq_t[:, t * 512 + a * 128: t * 512 + (a + 1) * 128],
                            xt[:, xcol(j, t):xcol(j, t) + CH],
                            start=(t == 0), stop=(t == NT - 1))
                    nc.scalar.activation(qt2[a][:, j * CH:(j + 1) * CH],
                                         psq[:],
                                         mybir.ActivationFunctionType.Copy)

            # ---- attention (software-pipelined emission) ------------------
            EXP = mybir.ActivationFunctionType.Exp
            CPY = mybir.ActivationFunctionType.Copy

            def attention_chunk(ci):
                n0 = ci * CH
                ans = []
                for a in range(4):
                    pa0 = ps.tile([128, CH], f32, tag="av", bufs=2, name="pa0")
                    pa1 = ps.tile([128, CH], f32, tag="av", bufs=2, name="pa1")
                    scnt = [0]

                    def emit_scores(kind, idx):
                        if kind == "off":
                            mt, w, q0 = idx, CH, n0
                        else:
                            mt = 4 * ci + idx
                            w = (4 - idx) * 128
                            q0 = n0 + idx * 128
                        stag, sbufs = (("s", 4) if scnt[0] % 3 < 2
                                       else ("mm", 2))
                        scnt[0] += 1
                        ss0 = ps.tile([128, CH], f32, tag=stag, bufs=sbufs,
                                      name="ss0")
                        ss1 = ps.tile([128, CH], f32, tag=stag, bufs=sbufs,
                                      name="ss1")
                        nc.tensor.matmul(
                            ss0[:, 0:w], kt2[0:64, mt * 128:(mt + 1) * 128],
                            qt2[a][0:64, q0:n0 + CH], start=True, stop=True)
                        nc.tensor.matmul(
                            ss1[:, 0:w], kt2[64:128, mt * 128:(mt + 1) * 128],
                            qt2[a][64:128, q0:n0 + CH], start=True, stop=True)
                        pt0 = wpool.tile([128, w], bf16, tag="pt", bufs=12,
                                         name="pt0")
                        pt1 = wpool.tile([128, w], bf16, tag="pt", bufs=12,
                                         name="pt1")
                        nc.scalar.activation(pt0[:], ss0[:, 0:w], EXP,
                                             scale=0.125)
                        nc.scalar.activation(pt1[:], ss1[:, 0:w], EXP,
                                             scale=0.125)
                        if kind == "diag":
                            nc.vector.tensor_mul(pt0[:, 0:128], pt0[:, 0:128],
                                                 msk_t[:])
                            nc.vector.tensor_mul(pt1[:, 0:128], pt1[:, 0:128],
                                                 msk_t[:])
                        return (kind, idx, mt, pt0, pt1)

                    def emit_av(sd):
                        kind, idx, mt, pt0, pt1 = sd
                        if kind == "off":
                            first, stop, c0 = (mt == 0), False, 0
                        else:
                            first = (ci == 0 and idx == 0)
                            stop, c0 = (idx == 3), idx * 128
                        nc.tensor.matmul(
                            pa0[:, c0:CH], v3[:, mt * 256: mt * 256 + 128],
                            pt0[:], start=first, stop=stop,
                            skip_group_check=True)
                        nc.tensor.matmul(
                            pa1[:, c0:CH],
                            v3[:, mt * 256 + 128: mt * 256 + 256],
                            pt1[:], start=first, stop=stop,
                            skip_group_check=True)

                    steps = ([("off", mt) for mt in range(4 * ci)]
                             + [("diag", s) for s in range(4)])
                    pend = []
                    for kind, idx in steps:
                        sd = emit_scores(kind, idx)
                        pend.append(sd)
                        if len(pend) > 2 and pend[0][0] == "off":
                            emit_av(pend.pop(0))
                    for sd in pend:
                        emit_av(sd)
                    # normalize: rows 64:128 of pa hold the softmax denom
                    # (replicated x64 by the ones block in v3).  Act stages
                    # it to SBUF; the approx reciprocal (~18 bits, plenty
                    # for the bf16 multiply; d>=1 so no edge cases) is a
                    # single DVE op.
                    dd0 = spool.tile([64, CH], f32, tag="dd")
                    dd1 = spool.tile([64, CH], f32, tag="dd")
                    nc.scalar.activation(dd0[:], pa0[64:128, :], CPY)
                    nc.scalar.activation(dd1[:], pa1[64:128, :], CPY)
                    rbr0 = spool.tile([64, CH], f32, tag="rbr")
                    rbr1 = spool.tile([64, CH], f32, tag="rbr")
                    nc.vector.reciprocal_approx_fast(rbr0[:], dd0[:])
                    nc.vector.reciprocal_approx_fast(rbr1[:], dd1[:])
                    an = apool.tile([128, CH], bf16, tag=f"an_{a}", bufs=2,
                                    name=f"an_{a}")
                    nc.vector.tensor_mul(an[0:64, :], pa0[0:64, :], rbr0[:])
                    nc.vector.tensor_mul(an[64:128, :], pa1[0:64, :], rbr1[:])
                    ans.append(an)
                return ans

            def oproj_chunk(ci, ans):
                # o_proj: out[n, :] += sum_c attn_outT_s[c, n] * Wo[c, :]
                n0 = ci * CH
                for nt in range(4):
                    for dc in range(4):
                        po = ps.tile([128, CH], f32, tag="mm", bufs=2,
                                     name="po")
                        for a in range(4):
                            nc.tensor.matmul(
                                po[:], ans[a][:, nt * 128:(nt + 1) * 128],
                                wo_t[:, a * D + dc * CH: a * D + (dc + 1) * CH],
                                start=(a == 0), stop=(a == 3))
                        st = spool.tile([128, CH], f32, tag="ost")
                        nc.scalar.activation(st[:], po[:], CPY)
                        nc.sync.dma_start(
                            part.ap()[n0 + nt * 128: n0 + (nt + 1) * 128,
                                      dc * CH:(dc + 1) * CH],
                            st[:])

            # ---- merged schedule: chase the DMA, start every engine early
            anss = {}
            for j in range(NCHUNK):
                kv_chunk(j)
                q_chunk(j)
                anss[j] = attention_chunk(j)
                if j >= 1:
                    oproj_chunk(j - 1, anss.pop(j - 1))
            oproj_chunk(NCHUNK - 1, anss.pop(NCHUNK - 1))
    nc.compile()
    return nc


def _tile128(a):
    """[K*128, M] -> [128, K*M], k-tile-major per partition."""
    k = a.shape[0] // 128
    return np.ascontiguousarray(
        a.reshape(k, 128, a.shape[1]).transpose(1, 0, 2).reshape(128, -1))


def _prep_in_maps(x, Wq, Wk, Wv, Wo):
    import jax.numpy as jnp

    def to_bf16(a):
        return np.asarray(jnp.asarray(np.asarray(a), dtype=jnp.bfloat16))

    # triangular causal mask for a 128x128 diagonal block (key i <= query j)
    i = np.arange(128)[:, None]
    j = np.arange(128)[None, :]
    msk = (i <= j).astype(np.float32)
    iden = np.eye(128, dtype=np.float32)

    in_maps = []
    for c in range(N_CORES):
        b, g = c // 4, c % 4
        qh = [8 * g + a for a in range(8)]      # global q heads for this core
        # Wq columns reordered into pair chunks [head a | head a+4]
        wq_cols = []
        for a in range(4):
            wq_cols.append(np.arange(qh[a] * HD, (qh[a] + 1) * HD))
            wq_cols.append(np.arange(qh[a + 4] * HD, (qh[a + 4] + 1) * HD))
        wq_r = np.asarray(Wq)[:, np.concatenate(wq_cols)]
        wo_rows = wq_cols  # same ordering for Wo rows
        wo_r = np.asarray(Wo)[np.concatenate(wo_rows), :]
        wk_s = np.asarray(Wk)[:, 2 * g * HD: (2 * g + 2) * HD]
        wv_s = np.asarray(Wv)[:, 2 * g * HD: (2 * g + 2) * HD]
        # x pre-tiled chunk-major: [t, p, j, n] -> [p, (j, t, n)]
        xb = np.asarray(x)[b].T  # [D, N]
        xc = np.ascontiguousarray(
            xb.reshape(NT, 128, NCHUNK, CH).transpose(1, 2, 0, 3)
            .reshape(128, -1))
        in_maps.append({
            "xT": to_bf16(xc),
            "wq": to_bf16(_tile128(wq_r)),
            "wk": to_bf16(_tile128(wk_s)),
            "wv": to_bf16(_tile128(wv_s)),
            "wo": to_bf16(_tile128(wo_r)),
            "msk": to_bf16(msk),
            "iden": to_bf16(iden),
        })
    return in_maps


def kernel(x, Wq, Wk, Wv, Wo, trace=False):
    if "nc" not in _CACHE:
        _CACHE["nc"] = _build()
    nc = _CACHE["nc"]
    in_maps = _prep_in_maps(x, Wq, Wk, Wv, Wo)
    res = bass_utils.run_bass_kernel_spmd(
        nc, in_maps, core_ids=list(range(N_CORES)), trace=trace)
    _CACHE["last_result"] = res
    out = np.zeros((B, N, D), np.float32)
    for c in range(N_CORES):
        out[c // 4] += res.results[c]["part"]
    return out
